# revision 9
# baseline (speedup 1.0000x reference)
"""Tensor-parallel GQA attention block (AtlasAttentionWrapper) on 8 TRN2 cores.

Sharding: TP over heads. Core m owns query heads [4m..4m+3] (Wq rows
m*512:(m+1)*512), KV head m (Wk/Wv rows m*128:(m+1)*128, past_k/past_v head m)
and Wo columns m*512:(m+1)*512. Each core computes a full [1024, 4096] o_proj
partial; a ReduceScatter sums them and leaves rows [128m:128(m+1)] on core m
(in 4 interleaved chunks); the host reassembles.

All device inputs are host-packed into SBUF layout [128, *] so every DMA is a
single large transfer with long contiguous per-partition lines:
  xTp   [128, 32*1024]   xTp[p, c*1024+s] = x[s, 128c+p]
  wqp   [128, 32*512]    wqp[p, c*512+o]  = Wq_shard[o, 128c+p]  (o in [0,512))
  wkp   [128, 32*128]    wkp[p, c*128+d]  = Wk_shard[d, 128c+p]
  wvp   [128, 32*128]    same for Wv
  wop   [128, 4*4096]    wop[p, j*4096+h] = Wo[h, 128j+p + 512m]
  pkT   [128, 1024]      roped past_k^T (d, s)
  pvp   [128, 8*128]     pvp[p, c*128+d] = past_v[128c+p, d]
  cosk/sink [128, 2048]  rope tables^T for all kv positions

Phase 1 runs chunk-outer with persistent PSUM accumulators so the PE starts
as soon as the first weight/xT pieces land: pass A accumulates K, V, Q0
(6 banks), pass B accumulates Q1-3 (6 banks).  Ropes/V-transposes overlap
pass B on DVE/Scalar.  Attention (scores built transposed, exp on scalar with
2-chunk batching, softmax denominator via ones-matmul) and o_proj + chunked
ReduceScatter follow, mutually interleaved as in the baseline.
"""

import sys

if "/opt/trn_rl_repo" not in sys.path:
    sys.path.insert(0, "/opt/trn_rl_repo")

from contextlib import ExitStack

import ml_dtypes
import numpy as np

import concourse.bass as bass
import concourse.tile as tile
from concourse import bacc, mybir
from concourse.bass import ds, ts
from concourse.bass_utils import run_bass_kernel_spmd
from concourse.masks import make_identity

NCORES = 8
B, SQ, H = 1, 1024, 4096
NH, NKV, D = 32, 8, 128
SP = 1024
KV = SP + SQ  # 2048
HPC = NH // NCORES  # 4 query heads per core
DQ = HPC * D  # 512
SH = SQ // NCORES  # 128 output rows per core after ReduceScatter
ROPE_THETA = 10000.0
INV_SQRT_D = 1.0 / float(np.sqrt(D))

BF16 = mybir.dt.bfloat16
F32 = mybir.dt.float32
HCH = H // 128  # 32 contraction chunks
KVCH = KV // 128  # 16 kv chunks
NCHUNK = 4  # ReduceScatter chunks (overlap comm with o_proj)
EXP = mybir.ActivationFunctionType.Exp

LAST_RESULT = None
_NC_CACHE = {}


def _rope_write(nc, tmp_pool, dst, src, cos_sb, sin_sb, pos, width):
    """dst[d, s] = rope(src)[d, s] for s in [pos, pos+width) absolute positions.

    src: AP [128, width] (PSUM f32 or SBUF bf16), dst: SBUF bf16 AP.
    rope: out[d<64] = x[d]*cos[d] - x[d+64]*sin[d]
          out[d>=64] = x[d]*cos[d] + x[d-64]*sin[d]
    """
    cs = cos_sb[:, ds(pos, width)]
    sn = sin_sb[:, ds(pos, width)]
    t = tmp_pool.tile([128, width], F32, tag="rope_t")
    u = tmp_pool.tile([128, width], F32, tag="rope_u")
    nc.vector.tensor_mul(t[0:64, :], src[64:128, :], sn[0:64, :])
    nc.vector.tensor_mul(t[64:128, :], src[0:64, :], sn[64:128, :])
    nc.vector.tensor_mul(u[:, :], src[:, :], cs)
    nc.vector.tensor_sub(dst[0:64, :], u[0:64, :], t[0:64, :])
    nc.vector.tensor_add(dst[64:128, :], u[64:128, :], t[64:128, :])


def _build_nc():
    nc = bacc.Bacc(None, target_bir_lowering=False, debug=False)

    xTp = nc.declare_dram_parameter("xTp", [128, HCH, SQ], BF16, False)
    wqp = nc.declare_dram_parameter("wqp", [128, HCH, DQ], BF16, False)
    wkp = nc.declare_dram_parameter("wkp", [128, HCH, D], BF16, False)
    wvp = nc.declare_dram_parameter("wvp", [128, HCH, D], BF16, False)
    wop = nc.declare_dram_parameter("wop", [128, HPC, H], BF16, False)
    pkT = nc.declare_dram_parameter("pkT", [D, SP], BF16, False)
    pvp = nc.declare_dram_parameter("pvp", [128, SP // 128, D], BF16, False)
    cosk = nc.declare_dram_parameter("cosk", [D, KV], BF16, False)
    sink = nc.declare_dram_parameter("sink", [D, KV], BF16, False)
    out_ext = nc.declare_dram_parameter("out", [SH, H], BF16, True)

    with tile.TileContext(nc) as tc, ExitStack() as ctx:
        # ---- persistent SBUF residents (live across all phases)
        const = ctx.enter_context(tc.tile_pool(name="const", bufs=1))
        kT_sb = const.tile([128, KV], BF16)  # roped K^T  [d, kv]
        v_sb = const.tile([128, KVCH, D], BF16)  # V chunks [kv%128, chunk, d]
        qT_sb = const.tile([128, HPC, SQ], BF16)  # roped Q^T per head [d, h, s]
        attnT_sb = const.tile([128, HPC, SQ], BF16)  # attn^T [d, h, s]
        cos_sb = const.tile([128, KV], BF16)
        sin_sb = const.tile([128, KV], BF16)
        ident = const.tile([128, 128], BF16)
        ones_sb = const.tile([128, 128], BF16)

        # small loads on the scalar queue (sync queue is reserved for the
        # weight/xT stream that gates the first matmuls)
        nc.scalar.dma_start(out=cos_sb[:, :], in_=cosk[:, :])
        nc.scalar.dma_start(out=sin_sb[:, :], in_=sink[:, :])
        nc.scalar.dma_start(out=v_sb[:, 0 : SP // 128, :], in_=pvp[:, :, :])
        nc.scalar.dma_start(out=kT_sb[:, 0:SP], in_=pkT[:, :])
        make_identity(nc, ident[:, :])
        nc.vector.memset(ones_sb[:, :], 1.0)

        rope_tmp = ctx.enter_context(tc.tile_pool(name="rope_tmp", bufs=2))
        dram = ctx.enter_context(tc.tile_pool(name="dram", bufs=1, space="DRAM"))
        part_chunks = []
        rs_chunks = []
        for k in range(NCHUNK):
            part_chunks.append(
                dram.tile([SQ // NCHUNK, H], BF16, tag=f"part{k}", name=f"part{k}")
            )
            rs_chunks.append(
                dram.tile(
                    [SQ // NCHUNK // NCORES, H], BF16, tag=f"rs{k}", name=f"rs{k}"
                )
            )

        # ================= Phase 1: projections + rope ==================
        with tc.tile_pool(name="proj", bufs=1) as proj, tc.tile_pool(
            name="ph1_ps", bufs=1, space="PSUM"
        ) as ph1_ps, tc.tile_pool(name="tr_ps", bufs=2, space="PSUM") as tr_ps:
            xT_sb = proj.tile([128, HCH, SQ], BF16)
            wqT_sb = proj.tile([128, HCH, DQ], BF16)
            wkT_sb = proj.tile([128, HCH, D], BF16)
            wvT_sb = proj.tile([128, HCH, D], BF16)

            xr, wqr, wkr, wvr = xTp, wqp, wkp, wvp

            # Weight/xT stream on the sync queue, ordered so the chunk-outer
            # pass A (K, V, Q0 accumulation) never waits on DMA.
            nc.sync.dma_start(out=wkT_sb[:, 0:16, :], in_=wkr[:, 0:16, :])
            nc.sync.dma_start(out=xT_sb[:, 0:2, :], in_=xr[:, 0:2, :])
            nc.sync.dma_start(out=wvT_sb[:, 0:16, :], in_=wvr[:, 0:16, :])
            nc.sync.dma_start(out=xT_sb[:, 2:4, :], in_=xr[:, 2:4, :])
            # Q0 weights (wq chunk columns 0:128) ride the scalar queue (small)
            nc.scalar.dma_start(out=wqT_sb[:, 0:16, 0:128], in_=wqr[:, 0:16, 0:128])
            nc.sync.dma_start(out=xT_sb[:, 4:6, :], in_=xr[:, 4:6, :])
            nc.sync.dma_start(out=xT_sb[:, 6:8, :], in_=xr[:, 6:8, :])
            nc.sync.dma_start(out=wkT_sb[:, 16:32, :], in_=wkr[:, 16:32, :])
            nc.sync.dma_start(out=xT_sb[:, 8:12, :], in_=xr[:, 8:12, :])
            nc.sync.dma_start(out=wvT_sb[:, 16:32, :], in_=wvr[:, 16:32, :])
            nc.scalar.dma_start(
                out=wqT_sb[:, 16:32, 0:128], in_=wqr[:, 16:32, 0:128]
            )
            nc.sync.dma_start(out=xT_sb[:, 12:16, :], in_=xr[:, 12:16, :])
            nc.sync.dma_start(out=xT_sb[:, 16:24, :], in_=xr[:, 16:24, :])
            nc.sync.dma_start(out=xT_sb[:, 24:32, :], in_=xr[:, 24:32, :])
            # Q1-3 weights, needed only when pass B starts (~50us in)
            nc.scalar.dma_start(out=wqT_sb[:, :, 128:512], in_=wqr[:, :, 128:512])

            # ---- pass A: chunk-outer accumulation of K, V, Q0 (6 banks)
            kps = ph1_ps.tile([128, 2, 512], F32, tag="pa0")
            vps = ph1_ps.tile([128, 2, 512], F32, tag="pa1")
            q0ps = ph1_ps.tile([128, 2, 512], F32, tag="pa2")
            for c in range(HCH):
                st_flags = dict(start=(c == 0), stop=(c == HCH - 1))
                for g in range(2):
                    nc.tensor.matmul(
                        kps[:, g, :],
                        lhsT=wkT_sb[:, c, :],
                        rhs=xT_sb[:, c, ts(g, 512)],
                        **st_flags,
                    )
                    nc.tensor.matmul(
                        vps[:, g, :],
                        lhsT=wvT_sb[:, c, :],
                        rhs=xT_sb[:, c, ts(g, 512)],
                        **st_flags,
                    )
                    nc.tensor.matmul(
                        q0ps[:, g, :],
                        lhsT=wqT_sb[:, c, 0:128],
                        rhs=xT_sb[:, c, ts(g, 512)],
                        **st_flags,
                    )

            # K rope + Q0 rope on DVE; V copy on scalar (idle in phase 1)
            vt_sb = proj.tile([128, 2, 512], BF16)
            nc.scalar.activation(
                vt_sb[:, :, :], vps[:, :, :], mybir.ActivationFunctionType.Copy
            )
            for g in range(2):
                _rope_write(
                    nc, rope_tmp, kT_sb[:, ds(SP + g * 512, 512)], kps[:, g, :],
                    cos_sb, sin_sb, SP + g * 512, 512,
                )
                _rope_write(
                    nc, rope_tmp, qT_sb[:, 0, ts(g, 512)], q0ps[:, g, :],
                    cos_sb, sin_sb, SP + g * 512, 512,
                )
            # V transposes into v_sb chunks [SP/128 ..) (PE, 2 spare banks)
            for k in range(8):
                ps2 = tr_ps.tile([128, 128], BF16, tag="tr")
                nc.tensor.transpose(
                    ps2[:, :], vt_sb[:, k // 4, ts(k % 4, 128)], ident[:, :]
                )
                nc.scalar.copy(v_sb[:, SP // 128 + k, :], ps2[:, :])

            # ---- pass B: chunk-outer accumulation of Q1-3 (6 banks)
            q1ps = ph1_ps.tile([128, 2, 512], F32, tag="pa0")
            q2ps = ph1_ps.tile([128, 2, 512], F32, tag="pa1")
            q3ps = ph1_ps.tile([128, 2, 512], F32, tag="pa2")
            for c in range(HCH):
                st_flags = dict(start=(c == 0), stop=(c == HCH - 1))
                for g in range(2):
                    for j, qps in ((1, q1ps), (2, q2ps), (3, q3ps)):
                        nc.tensor.matmul(
                            qps[:, g, :],
                            lhsT=wqT_sb[:, c, ts(j, 128)],
                            rhs=xT_sb[:, c, ts(g, 512)],
                            **st_flags,
                        )
            for j, qps in ((1, q1ps), (2, q2ps), (3, q3ps)):
                for g in range(2):
                    _rope_write(
                        nc, rope_tmp, qT_sb[:, j, ts(g, 512)], qps[:, g, :],
                        cos_sb, sin_sb, SP + g * 512, 512,
                    )

        # ============ Phase 2+3 interleaved: attention, o_proj, RS ==========
        # Attention runs g-outer (all heads for q-half g), so the o_proj +
        # ReduceScatter of q-half 0 overlaps the attention of q-half 1.
        st_ps = ctx.enter_context(tc.tile_pool(name="st_ps", bufs=2, space="PSUM"))
        sums_ps = ctx.enter_context(
            tc.tile_pool(name="sums_ps", bufs=2, space="PSUM")
        )
        at_ps = ctx.enter_context(tc.tile_pool(name="at_ps", bufs=2, space="PSUM"))

        pt_pool = ctx.enter_context(tc.tile_pool(name="pt", bufs=4))
        rc_pool = ctx.enter_context(tc.tile_pool(name="rc", bufs=2))
        wo_pool = ctx.enter_context(tc.tile_pool(name="wo", bufs=1))
        ob_pool = ctx.enter_context(tc.tile_pool(name="ob", bufs=3))
        wo_sb = wo_pool.tile([128, HPC, H], BF16)
        nc.scalar.dma_start(out=wo_sb[:, :, :], in_=wop[:, :, :])

        spc = SQ // NCHUNK // 128  # s-tiles per RS chunk
        rsh = SQ // NCHUNK // NCORES  # rows per core per RS chunk

        def attention_half(g):
            for h in range(HPC):
                sums = sums_ps.tile([128, 512], F32, tag="sums", name=f"sums{h}{g}")
                att = at_ps.tile([128, 512], F32, tag="att", name=f"att{h}{g}")
                for cc in range(KVCH // 2):
                    st = st_ps.tile([128, 2, 512], F32, tag="st", name="st")
                    pt = pt_pool.tile([128, 2, 512], BF16, name="pt")
                    for j in range(2):
                        nc.tensor.matmul(
                            st[:, j, :],
                            lhsT=kT_sb[:, ts(2 * cc + j, 128)],
                            rhs=qT_sb[:, h, ts(g, 512)],
                            start=True,
                            stop=True,
                        )
                    nc.scalar.activation(
                        pt[:, :, :], st[:, :, :], EXP, scale=INV_SQRT_D
                    )
                    for j in range(2):
                        c = 2 * cc + j
                        nc.tensor.matmul(
                            sums[:, :],
                            lhsT=ones_sb[:, :],
                            rhs=pt[:, j, :],
                            start=(c == 0),
                            stop=(c == KVCH - 1),
                        )
                        nc.tensor.matmul(
                            att[:, :],
                            lhsT=v_sb[:, c, :],
                            rhs=pt[:, j, :],
                            start=(c == 0),
                            stop=(c == KVCH - 1),
                        )
                recip = rc_pool.tile([128, 512], F32, name="recip")
                nc.vector.reciprocal_approx_fast(recip[:, :], sums[:, :])
                nc.vector.tensor_mul(
                    attnT_sb[:, h, ts(g, 512)], att[:, :], recip[:, :]
                )

        def oproj_chunk(k):
            for ii in range(spc):
                i = k * spc + ii
                for nb in range(H // 1024):
                    ps = st_ps.tile([128, 2, 512], F32, tag="st", name="ops")
                    ob = ob_pool.tile([128, 2, 512], BF16, name="ob")
                    for half in range(2):
                        n = 2 * nb + half
                        for j in range(HPC):
                            nc.tensor.matmul(
                                ps[:, half, :],
                                lhsT=attnT_sb[:, j, ts(i, 128)],
                                rhs=wo_sb[:, j, ts(n, 512)],
                                start=(j == 0),
                                stop=(j == HPC - 1),
                            )
                    nc.vector.tensor_copy(ob[:, :, :], ps[:, :, :])
                    nc.sync.dma_start(
                        out=part_chunks[k][ts(ii, 128), ts(nb, 1024)],
                        in_=ob[:, :, :],
                    )
            nc.gpsimd.collective_compute(
                "ReduceScatter",
                mybir.AluOpType.add,
                ins=[part_chunks[k][:, :].opt()],
                outs=[rs_chunks[k][:, :].opt()],
                replica_groups=[list(range(NCORES))],
            )
            nc.sync.dma_start(out=out_ext[ts(k, rsh), :], in_=rs_chunks[k][:, :])

        attention_half(0)
        for k in range(NCHUNK // 2):
            oproj_chunk(k)
        attention_half(1)
        for k in range(NCHUNK // 2, NCHUNK):
            oproj_chunk(k)

    nc.finalize()
    return nc


def _get_nc():
    if "nc" not in _NC_CACHE:
        _NC_CACHE["nc"] = _build_nc()
    return _NC_CACHE["nc"]


def _rope_tables():
    inv_freq = 1.0 / (ROPE_THETA ** (np.arange(0, D, 2, dtype=np.float32) / D))
    pos = np.arange(KV, dtype=np.float32)
    freqs = pos[:, None] * inv_freq[None, :]  # [KV, D/2]
    emb = np.concatenate([freqs, freqs], axis=-1)  # [KV, D]
    return np.cos(emb), np.sin(emb)  # [KV, D]


def _host_rope(x, cos, sin):
    # x: [S, D]; cos/sin: [S, D]
    x1, x2 = x[:, : D // 2], x[:, D // 2 :]
    rot = np.concatenate([-x2, x1], axis=-1)
    return x * cos + rot * sin


def _pack(mat_t, inner):
    """[n*128, inner] -> [128, n, inner]: SBUF layout, partition dim first."""
    n = mat_t.shape[0] // 128
    return np.ascontiguousarray(mat_t.reshape(n, 128, inner).transpose(1, 0, 2))


def kernel(hidden_states, past_k, past_v, Wq, Wk, Wv, Wo, trace=False):
    global LAST_RESULT
    bf = ml_dtypes.bfloat16
    x = np.asarray(hidden_states, dtype=np.float32)[0]  # [SQ, H]
    xTp = _pack(np.ascontiguousarray(x.T), SQ).astype(bf)
    cos, sin = _rope_tables()  # [KV, D] f32
    cosT = np.ascontiguousarray(cos.T).astype(bf)
    sinT = np.ascontiguousarray(sin.T).astype(bf)

    in_maps = []
    for m in range(NCORES):
        qr = slice(m * DQ, (m + 1) * DQ)
        kr = slice(m * D, (m + 1) * D)
        in_maps.append(
            {
                "xTp": xTp,
                "wqp": _pack(np.asarray(Wq)[qr].T, DQ).astype(bf),
                "wkp": _pack(np.asarray(Wk)[kr].T, D).astype(bf),
                "wvp": _pack(np.asarray(Wv)[kr].T, D).astype(bf),
                "wop": _pack(np.asarray(Wo)[:, qr].T, H).astype(bf),
                "pkT": np.ascontiguousarray(
                    _host_rope(
                        np.asarray(past_k, dtype=np.float32)[0, m], cos[:SP], sin[:SP]
                    ).T
                ).astype(bf),
                "pvp": _pack(np.asarray(past_v)[0, m], D).astype(bf),
                "cosk": cosT,
                "sink": sinT,
            }
        )

    nc = _get_nc()
    res = run_bass_kernel_spmd(
        nc, in_maps, core_ids=list(range(NCORES)), trace=trace
    )
    LAST_RESULT = res
    # Each core's "out" holds NCHUNK blocks of rsh rows; block k of core m is
    # global rows [csz*k + rsh*m, csz*k + rsh*(m+1)).
    csz = SQ // NCHUNK
    rsh = csz // NCORES
    out = np.empty((SQ, H), dtype=np.float32)
    for m in range(NCORES):
        shard = np.asarray(res.results[m]["out"], dtype=np.float32)
        for k in range(NCHUNK):
            out[csz * k + rsh * m : csz * k + rsh * (m + 1)] = shard[
                rsh * k : rsh * (k + 1)
            ]
    return out.reshape(B, SQ, H)
